# revision 3
# baseline (speedup 1.0000x reference)
"""DiffAttn Trainium2 kernel (8-core SPMD, no collectives) — v2.

Problem: B=2, T=2048, IN_DIM=OUT_DIM=1024, H=8 v-heads (2D=64), 2H=16 qk-heads
(D=32). Core c = 4*b + g handles batch b, head-group g: qk-heads {4g..4g+3}
(local heads h=0..3; h even = positive softmax, h odd = negative), v-heads
{2g, 2g+1}. Out-projection row-sharded; host sums 4 partials per batch.

v2 structure (vs v1): the dots PSUM is split pos/neg — heads {0,2} land in P,
heads {1,3} in N. exp(P) runs on the Scalar engine (ACT); exp(N) is computed
on the Vector engine with a Schraudolph bit-trick (one tensor_scalar:
int16(A*z+B) whose bits ARE the bf16 of e^z). The sawtooth error of the
approximation only touches the negative softmax, which the end-to-end test
shows is attenuated by the lambda-weighted subtraction (~3e-3 final rel err).
This halves the ACT workload, which was the phase-B critical path, and makes
the PE dense enough to hold its high clock state (HAM).

The per-query-block combine (r = lam*s1/s2, comb = a1 - r*a2, sumsq) runs
interleaved with the next block's attention, using the otherwise-idle GpSimd
engine for partition broadcasts / reductions and elementwise work. The RMS
sqrt is batched into the tail so the ACT exp table set is never swapped
mid-attention. gamma*(1-lambda_init) is folded into Wout on the host; the
1/rms scale commutes onto comb before the out-projection.
"""
import math

import numpy as np

H = 8
D = 32
LAMBDA_INIT = 0.8 - 0.6 * math.exp(-0.3)
B, T, IN_DIM, OUT_DIM = 2, 2048, 1024, 1024
E = 2 * H * D  # 512

N_CORES = 8
GROUPS = 4            # head groups (cores per batch)
QB = 512              # query block (matmul free dim)
NQB = T // QB         # 4
KT = 128              # key tile (partition dim)
NKT = T // KT         # 16
NIN = IN_DIM // 128   # 8

# Schraudolph exp in bf16-bit-space: bf16(e^z) ~= bits(int16(A16*z + B16)).
# B16 tuned for ~zero mean rel err; constant offset cancels in softmax.
A16 = 128.0 / math.log(2.0)
B16 = 1064866805.0 / 65536.0

_compiled = None


def _build():
    import concourse.bass as bass  # noqa: F401
    import concourse.mybir as mybir
    from concourse import bacc, bass_isa
    from concourse.tile import TileContext

    f32 = mybir.dt.float32
    bf16 = mybir.dt.bfloat16
    i16 = mybir.dt.int16
    AF = mybir.ActivationFunctionType
    MUL = mybir.AluOpType.mult
    ADD = mybir.AluOpType.add

    nc = bacc.Bacc("TRN2", target_bir_lowering=False, num_devices=N_CORES)

    xT = nc.dram_tensor("xT", [IN_DIM, T], bf16, kind="ExternalInput")
    wq = nc.dram_tensor("wq", [IN_DIM, 128], bf16, kind="ExternalInput")
    wk = nc.dram_tensor("wk", [IN_DIM, 128], bf16, kind="ExternalInput")
    wv = nc.dram_tensor("wv", [IN_DIM, 128], bf16, kind="ExternalInput")
    wo0 = nc.dram_tensor("wo0", [64, OUT_DIM], bf16, kind="ExternalInput")
    wo1 = nc.dram_tensor("wo1", [64, OUT_DIM], bf16, kind="ExternalInput")
    lam = nc.dram_tensor("lam", [128, 1], f32, kind="ExternalInput")
    outT = nc.dram_tensor("outT", [OUT_DIM, T], f32, kind="ExternalOutput")

    with TileContext(nc) as tc:
        with tc.tile_pool(name="persist", bufs=1) as pp:
            # ---- persistent SBUF ----
            wq_sb = pp.tile([128, NIN, 128], bf16)
            wk_sb = pp.tile([128, NIN, 128], bf16)
            wv_sb = pp.tile([128, NIN, 128], bf16)
            wo0_sb = pp.tile([64, OUT_DIM], bf16)
            wo1_sb = pp.tile([64, OUT_DIM], bf16)
            qT_sb = pp.tile([128, T], bf16)          # 4 qk-heads x 32 rows
            kT_sb = pp.tile([128, T], bf16)
            v_sb = pp.tile([128, NKT, 130], bf16)    # [t, kt, (vA|1|vB|1)]
            # staged attnv accumulators: rows 0-63 = v dims, row 64 = s row
            a1g = [pp.tile([65, T], f32, name=f"a1g{v}") for v in range(2)]
            a2g = [pp.tile([65, T], f32, name=f"a2g{v}") for v in range(2)]
            # combined (a1 - r*a2) pre-norm, bf16 for the out-projection
            cmb = [pp.tile([64, T], bf16, name=f"cmb{v}") for v in range(2)]
            finl = [pp.tile([64, T], bf16, name=f"finl{v}") for v in range(2)]
            # sumsq rows broadcast to 64 partitions (gpsimd all_reduce out)
            ssB = [pp.tile([64, T], f32, name=f"ssB{v}") for v in range(2)]
            # s rows fanned onto partitions 0-1 via DMA: row vh
            s1f = pp.tile([2, T], f32)
            s2f = pp.tile([2, T], f32)
            lam_sb = pp.tile([128, 1], f32)
            warm = pp.tile([128, 512], bf16)

            # ---- input DMAs ----
            nc.sync.dma_start(out=lam_sb[:, :], in_=lam[:, :])
            nc.sync.dma_start(out=wq_sb[:, :, :],
                              in_=wq.rearrange("(c p) m -> p c m", p=128))
            nc.sync.dma_start(out=wk_sb[:, :, :],
                              in_=wk.rearrange("(c p) m -> p c m", p=128))
            nc.sync.dma_start(out=wv_sb[:, :, :],
                              in_=wv.rearrange("(c p) m -> p c m", p=128))
            nc.sync.dma_start(out=wo0_sb[:, :], in_=wo0[:, :])
            nc.sync.dma_start(out=wo1_sb[:, :], in_=wo1[:, :])

            # ---- phase A: x^T load (per chunk), warmup, projections ----
            with (
                tc.tile_pool(name="xpool", bufs=1) as xp,
                tc.tile_pool(name="psA", bufs=2, space="PSUM") as psA,
            ):
                xT_sb = xp.tile([128, NIN, T], bf16)
                for c in range(NIN):
                    nc.sync.dma_start(out=xT_sb[:, c, :],
                                      in_=xT[128 * c:128 * (c + 1), :])

                nc.vector.memset(warm[:, :], 0.0)
                wm = psA.tile([128, 512], f32, tag="warm", bufs=1)
                for _ in range(24):
                    nc.tensor.matmul(wm[:, :], warm[:, :128], warm[:, :],
                                     start=True, stop=True)
                nc.scalar.activation(warm[:64, :], wm[:64, :], AF.Exp)

                for dst, w_sb in ((qT_sb, wq_sb), (kT_sb, wk_sb)):
                    for tb in range(NQB):
                        p = psA.tile([128, QB], f32, tag="proj")
                        for c in range(NIN):
                            nc.tensor.matmul(
                                p[:, :], w_sb[:, c, :],
                                xT_sb[:, c, tb * QB:(tb + 1) * QB],
                                start=(c == 0), stop=(c == NIN - 1))
                        nc.vector.tensor_copy(dst[:, tb * QB:(tb + 1) * QB],
                                              p[:, :])
                for kt in range(NKT):
                    p = psA.tile([128, 128], f32, tag="vproj")
                    for c in range(NIN):
                        nc.tensor.matmul(
                            p[:, :], xT_sb[:, c, kt * 128:(kt + 1) * 128],
                            wv_sb[:, c, :], start=(c == 0), stop=(c == NIN - 1))
                    nc.vector.tensor_copy(v_sb[:, kt, 0:64], p[:, 0:64])
                    nc.vector.tensor_copy(v_sb[:, kt, 65:129], p[:, 64:128])
                with tc.tile_pool(name="onescr", bufs=1) as op_:
                    oscr = op_.tile([128, NKT], f32)
                    nc.vector.memset(oscr[:, :], 1.0)
                    nc.vector.tensor_copy(
                        v_sb[:, :, 64:65].rearrange("p n 1 -> p n"),
                        oscr[:, :])
                    nc.vector.tensor_copy(
                        v_sb[:, :, 129:130].rearrange("p n 1 -> p n"),
                        oscr[:, :])

            # ---- phase B: attention, with per-qb combine interleaved ----
            with (
                tc.tile_pool(name="dots_ps", bufs=1, space="PSUM") as dps,
                tc.tile_pool(name="acc_ps", bufs=1, space="PSUM") as aps,
                tc.tile_pool(name="epool", bufs=3) as ep,
                tc.tile_pool(name="cpool", bufs=2) as cp,
            ):
                for qb in range(NQB):
                    qs = qb * QB
                    # accs[2*vh + s]: s=0 pos, s=1 neg; 65 rows (64 v + s row)
                    accs = [aps.tile([65, QB], f32, tag=f"acc{j}",
                                     name=f"acc{j}_{qb}") for j in range(4)]
                    es = {}
                    for kt in range(NKT):
                        # P gets heads 0,2 (pos); N gets heads 1,3 (neg)
                        pt = dps.tile([128, 2 * QB], f32, tag="dP",
                                      name=f"dP_{qb}_{kt}")
                        nt = dps.tile([128, 2 * QB], f32, tag="dN",
                                      name=f"dN_{qb}_{kt}")
                        for h in range(4):
                            dp = pt if h % 2 == 0 else nt
                            nc.tensor.matmul(
                                dp[:, (h // 2) * QB:(h // 2 + 1) * QB],
                                kT_sb[32 * h:32 * (h + 1),
                                      kt * KT:(kt + 1) * KT],
                                qT_sb[32 * h:32 * (h + 1), qs:qs + QB],
                                start=True, stop=True,
                                tile_position=(32 * h, 0))
                        eP = ep.tile([128, 2 * QB], bf16, tag="eP",
                                     name=f"eP_{qb}_{kt}")
                        nc.scalar.activation(eP[:, :], pt[:, :], AF.Exp)
                        eNi = ep.tile([128, 2 * QB], i16, tag="eN",
                                      name=f"eN_{qb}_{kt}")
                        nc.vector.tensor_scalar(
                            eNi[:, :], nt[:, :], A16, B16, op0=MUL, op1=ADD)
                        es[kt] = (eP, eNi.bitcast(bf16))
                        if kt > 0:
                            _attnv(nc, accs, es, v_sb, kt - 1, NKT)
                    _attnv(nc, accs, es, v_sb, NKT - 1, NKT)

                    # ---- per-qb epilogue (overlaps next qb's attention) ----
                    # stage accs to SBUF: pos rows on ACT, neg rows on DVE
                    for vh in range(2):
                        nc.scalar.activation(a1g[vh][:, qs:qs + QB],
                                             accs[2 * vh][:, :], AF.Copy)
                        nc.vector.tensor_copy(a2g[vh][:, qs:qs + QB],
                                              accs[2 * vh + 1][:, :])
                    # fan s rows onto partitions 0-1 (vh) via SBUF DMA
                    for vh in range(2):
                        nc.sync.dma_start(out=s1f[vh:vh + 1, qs:qs + QB],
                                          in_=a1g[vh][64:65, qs:qs + QB])
                        nc.sync.dma_start(out=s2f[vh:vh + 1, qs:qs + QB],
                                          in_=a2g[vh][64:65, qs:qs + QB])
                    # r = lam * s1 / s2 for both vh at once
                    rec2 = cp.tile([2, QB], f32, tag="rec2")
                    nc.vector.reciprocal_approx_fast(rec2[:, :],
                                                     s2f[:, qs:qs + QB])
                    r2t = cp.tile([2, QB], f32, tag="r2t")
                    nc.vector.scalar_tensor_tensor(
                        r2t[:, :], s1f[:, qs:qs + QB], lam_sb[0:2, 0:1],
                        rec2[:, :], op0=MUL, op1=MUL)
                    # gpsimd broadcast needs a partition-0 source; fan row 1
                    r2b = cp.tile([1, QB], f32, tag="r2b")
                    nc.sync.dma_start(out=r2b[0:1, :], in_=r2t[1:2, :])
                    for vh in range(2):
                        rb = cp.tile([64, QB], f32, tag=f"rb{vh}")
                        src = r2t[0:1, :] if vh == 0 else r2b[0:1, :]
                        nc.gpsimd.partition_broadcast(rb[:, :], src)
                        t2 = cp.tile([64, QB], f32, tag=f"t2{vh}")
                        nc.gpsimd.tensor_mul(t2[:, :],
                                             a2g[vh][0:64, qs:qs + QB],
                                             rb[:, :])
                        nc.gpsimd.tensor_sub(cmb[vh][:, qs:qs + QB],
                                             a1g[vh][0:64, qs:qs + QB],
                                             t2[:, :])
                        sqq = cp.tile([64, QB], bf16, tag=f"sq{vh}")
                        nc.gpsimd.tensor_mul(sqq[:, :],
                                             cmb[vh][:, qs:qs + QB],
                                             cmb[vh][:, qs:qs + QB])
                        nc.gpsimd.partition_all_reduce(
                            ssB[vh][:, qs:qs + QB], sqq[:, :], 64,
                            bass_isa.ReduceOp.add)

            # ---- phase C: rms scale + out-projection ----
            with (
                tc.tile_pool(name="psC", bufs=3, space="PSUM") as psC,
                tc.tile_pool(name="sbC", bufs=3) as sbC,
            ):
                # rn = 1/sqrt(ss/64); finl = cmb * rn  (per vh, full T)
                for vh in range(2):
                    sqr = sbC.tile([64, T], f32, tag=f"sqr{vh}", bufs=1)
                    nc.scalar.activation(sqr[:, :], ssB[vh][:, :], AF.Sqrt,
                                         scale=1.0 / 64.0)
                    rn = sbC.tile([64, T], f32, tag=f"rn{vh}", bufs=1)
                    nc.vector.reciprocal_approx_fast(rn[:, :], sqr[:, :])
                    nc.gpsimd.tensor_mul(finl[vh][:, :], cmb[vh][:, :],
                                         rn[:, :])
                for qb in range(NQB):
                    qs = qb * QB
                    for oc in range(OUT_DIM // 128):
                        p = psC.tile([128, QB], f32, tag="oproj")
                        nc.tensor.matmul(p[:, :],
                                         wo0_sb[:, oc * 128:(oc + 1) * 128],
                                         finl[0][:, qs:qs + QB],
                                         start=True, stop=False)
                        nc.tensor.matmul(p[:, :],
                                         wo1_sb[:, oc * 128:(oc + 1) * 128],
                                         finl[1][:, qs:qs + QB],
                                         start=False, stop=True)
                        o = sbC.tile([128, QB], f32, tag="ostage")
                        if oc % 2 == 0:
                            nc.vector.tensor_copy(o[:, :], p[:, :])
                        else:
                            nc.scalar.activation(o[:, :], p[:, :], AF.Copy)
                        nc.sync.dma_start(
                            out=outT[oc * 128:(oc + 1) * 128, qs:qs + QB],
                            in_=o[:, :])

    nc.compile()
    return nc


def _attnv(nc, accs, es, v_sb, kt, nkt):
    eP, eN = es[kt]
    for j in range(4):
        e = eP if j % 2 == 0 else eN
        ecol = (j // 2) * QB
        vcol = 65 * (j // 2)
        nc.tensor.matmul(
            accs[j][:, :], v_sb[:, kt, vcol:vcol + 65],
            e[:, ecol:ecol + QB],
            start=(kt == 0), stop=(kt == nkt - 1))


def _get_compiled():
    global _compiled
    if _compiled is None:
        _compiled = _build()
    return _compiled


def make_in_maps(x, Wq, Wkv, Wout, lambda_q1, lambda_k1, lambda_q2, lambda_k2,
                 gamma):
    import ml_dtypes
    bf = ml_dtypes.bfloat16
    x = np.asarray(x, dtype=np.float32)
    Wq = np.asarray(Wq, dtype=np.float32)
    Wkv = np.asarray(Wkv, dtype=np.float32)
    Wout = np.asarray(Wout, dtype=np.float32)
    lam_v = (math.exp(float(np.dot(lambda_q1, lambda_k1)))
             - math.exp(float(np.dot(lambda_q2, lambda_k2))) + LAMBDA_INIT)
    lam_arr = np.full((128, 1), lam_v, dtype=np.float32)
    # gamma * (1 - lambda_init) folded into the out-projection rows
    gam_f = (np.asarray(gamma, dtype=np.float32)
             * (1.0 - LAMBDA_INIT)).reshape(64, 1)
    Wq_s = (Wq * (D ** -0.5)).astype(np.float32)
    Wk = Wkv[:, :E]
    Wv = Wkv[:, E:]
    xT_all = [np.ascontiguousarray(x[b].T).astype(bf) for b in range(B)]
    in_maps = []
    for c in range(N_CORES):
        b, g = divmod(c, GROUPS)
        sl = slice(128 * g, 128 * (g + 1))
        wo0 = Wout[128 * g:128 * g + 64, :] * gam_f
        wo1 = Wout[128 * g + 64:128 * (g + 1), :] * gam_f
        in_maps.append({
            "xT": xT_all[b],
            "wq": np.ascontiguousarray(Wq_s[:, sl]).astype(bf),
            "wk": np.ascontiguousarray(Wk[:, sl]).astype(bf),
            "wv": np.ascontiguousarray(Wv[:, sl]).astype(bf),
            "wo0": np.ascontiguousarray(wo0).astype(bf),
            "wo1": np.ascontiguousarray(wo1).astype(bf),
            "lam": lam_arr,
        })
    return in_maps


def kernel(x, Wq, Wkv, Wout, lambda_q1, lambda_k1, lambda_q2, lambda_k2,
           gamma, _run_kw=None):
    import sys
    if "/opt/trn_rl_repo" not in sys.path:
        sys.path.insert(0, "/opt/trn_rl_repo")
    from concourse.bass_utils import run_bass_kernel_spmd

    nc = _get_compiled()
    in_maps = make_in_maps(x, Wq, Wkv, Wout, lambda_q1, lambda_k1,
                           lambda_q2, lambda_k2, gamma)
    res = run_bass_kernel_spmd(nc, in_maps, list(range(N_CORES)),
                               **(_run_kw or {}))
    out = np.zeros((B, T, OUT_DIM), dtype=np.float32)
    for c in range(N_CORES):
        out[c // GROUPS] += res.results[c]["outT"].T
    kernel.last_result = res
    return out
